# revision 3
# baseline (speedup 1.0000x reference)
"""MultiHeadAttention (B=4, L=2048, D=512, H=8) on 8 TRN2 NeuronCores.

Sharding: core c owns attention units n = 4c..4c+3 of the [B*H=32] unit axis
(input batch b1 = c//2, heads 4*(c%2)..4*(c%2)+3).  Each core:
  - projects its batch's q/k/v for its 4 heads (qh^T / kh^T computed directly
    in [dh, L] layout from feature-major x^T; vh in natural [L, dh] layout),
  - computes scores [q, k] on PE (K=64 contraction), exp on ACT with the
    softmax denominator via accum_out, normalizes with DVE tensor_scalar,
    writes its raw_att slice [4, 2048, 2048] straight to HBM,
  - PE-transposes att blocks to feed att^T into the att@V matmul, giving
    att_out^T [dh, L] per unit,
  - projects att_out^T against its Wo channel block (units of core c form
    merged channel block j=c for all 4 output batches), producing a full
    [8192, 512] partial of the output linear,
  - ReduceScatter(add) over the 8 cores hands each core its 1024 output rows,
    where residual (+bo folded in on host) and LayerNorm finish the job.
Host assembles raw_att and out from the per-core output buffers.
"""

import numpy as np

B, L, D, H = 4, 2048, 512, 8
DH = D // H
N_CORES = 8
HPC = 4            # heads per core
LN_EPS = 1e-5
ROWS = B * L       # 8192
RPC = ROWS // N_CORES  # 1024 out rows per core

_CACHE = {}


def _build():
    import concourse.bass as bass
    import concourse.mybir as mybir
    import concourse.tile as tile
    from concourse import bacc
    from concourse.masks import make_identity

    f32 = mybir.dt.float32
    AF = mybir.ActivationFunctionType
    Alu = mybir.AluOpType

    nc = bacc.Bacc("TRN2", target_bir_lowering=False, debug=False)

    xq = nc.dram_tensor("xq", [L, D], f32, kind="ExternalInput").ap()
    xk = nc.dram_tensor("xk", [L, D], f32, kind="ExternalInput").ap()
    xv = nc.dram_tensor("xv", [L, D], f32, kind="ExternalInput").ap()
    wqT = nc.dram_tensor("wqT", [D, HPC * DH], f32, kind="ExternalInput").ap()
    wkT = nc.dram_tensor("wkT", [D, HPC * DH], f32, kind="ExternalInput").ap()
    wvT = nc.dram_tensor("wvT", [D, HPC * DH], f32, kind="ExternalInput").ap()
    bq4 = nc.dram_tensor("bq4", [DH, HPC], f32, kind="ExternalInput").ap()
    bk4 = nc.dram_tensor("bk4", [DH, HPC], f32, kind="ExternalInput").ap()
    bv_b = nc.dram_tensor("bv_b", [128, HPC * DH], f32, kind="ExternalInput").ap()
    woT4 = nc.dram_tensor("woT4", [DH, D], f32, kind="ExternalInput").ap()
    resid = nc.dram_tensor("resid", [RPC, D], f32, kind="ExternalInput").ap()
    gam_b = nc.dram_tensor("gam_b", [128, D], f32, kind="ExternalInput").ap()
    bet_b = nc.dram_tensor("bet_b", [128, D], f32, kind="ExternalInput").ap()

    attp = nc.dram_tensor("attp", [HPC, L, L], f32, kind="ExternalOutput").ap()
    outp = nc.dram_tensor("outp", [RPC, D], f32, kind="ExternalOutput").ap()

    with tile.TileContext(nc) as tc:
        with (
            tc.tile_pool(name="const", bufs=1) as constp,
            tc.tile_pool(name="proj", bufs=1) as projp,
            tc.tile_pool(name="dram", bufs=1, space="DRAM") as dramp,
        ):
            ident = constp.tile([128, 128], f32)
            make_identity(nc, ident[:])

            w_sb = {}
            for name, src in (("q", wqT), ("k", wkT), ("v", wvT)):
                wt = constp.tile([128, 4 * 256], f32, name=f"w_{name}")
                for f in range(4):
                    nc.sync.dma_start(wt[:, 256 * f : 256 * (f + 1)],
                                      src[128 * f : 128 * (f + 1), :])
                w_sb[name] = wt
            bq_sb = constp.tile([DH, HPC], f32)
            bk_sb = constp.tile([DH, HPC], f32)
            bv_sb = constp.tile([128, 256], f32)
            wo_sb = constp.tile([DH, D], f32)
            gam_sb = constp.tile([128, D], f32)
            bet_sb = constp.tile([128, D], f32)
            nc.sync.dma_start(bq_sb[:], bq4[:])
            nc.sync.dma_start(bk_sb[:], bk4[:])
            nc.sync.dma_start(bv_sb[:], bv_b[:])
            nc.sync.dma_start(wo_sb[:], woT4[:])
            nc.sync.dma_start(gam_sb[:], gam_b[:])
            nc.sync.dma_start(bet_sb[:], bet_b[:])

            # persistent projection outputs
            qp = [projp.tile([128, L], f32, name=f"qp{i}") for i in range(2)]
            kp = [projp.tile([128, L], f32, name=f"kp{i}") for i in range(2)]
            vh_all = projp.tile([128, 16, 256], f32)

            # ---------- Phase A: x^T and projections ----------
            with (
                tc.tile_pool(name="xT", bufs=5) as xTp,
                tc.tile_pool(name="xld", bufs=3) as xldp,
                tc.tile_pool(name="tp_ps", bufs=4, space="PSUM") as tpp,
                tc.tile_pool(name="pj_ps", bufs=2, space="PSUM") as pjp,
                tc.tile_pool(name="pv_ps", bufs=2, space="PSUM") as pvp,
            ):
                for tname, x_in in (("q", xq), ("k", xk), ("v", xv)):
                    xT = [xTp.tile([128, L], f32, name=f"xT{f}", tag="xT")
                          for f in range(4)]
                    for lg in range(4):
                        tp = [tpp.tile([128, 512], f32, name=f"tp{f}", tag="tp")
                              for f in range(4)]
                        for li4 in range(4):
                            xld = xldp.tile([128, 512], f32, name="xld")
                            nc.sync.dma_start(
                                xld[:], x_in[512 * lg + 128 * li4 : 512 * lg + 128 * (li4 + 1), :])
                            for f in range(4):
                                nc.tensor.transpose(
                                    tp[f][:, 128 * li4 : 128 * (li4 + 1)],
                                    xld[:, 128 * f : 128 * (f + 1)], ident[:])
                        for f in range(4):
                            if f % 2 == 0:
                                nc.vector.tensor_copy(
                                    xT[f][:, 512 * lg : 512 * (lg + 1)], tp[f][:])
                            else:
                                nc.scalar.copy(
                                    xT[f][:, 512 * lg : 512 * (lg + 1)], tp[f][:])
                    if tname in ("q", "k"):
                        dst_pair = qp if tname == "q" else kp
                        bias_sb = bq_sb if tname == "q" else bk_sb
                        wt = w_sb[tname]
                        for h in range(4):
                            for lg in range(4):
                                ps = pjp.tile([DH, 512], f32, name="ps", tag="ps")
                                for f in range(4):
                                    nc.tensor.matmul(
                                        ps[:],
                                        wt[:, 256 * f + 64 * h : 256 * f + 64 * (h + 1)],
                                        xT[f][:, 512 * lg : 512 * (lg + 1)],
                                        start=(f == 0), stop=(f == 3))
                                nc.scalar.activation(
                                    dst_pair[h // 2][64 * (h % 2) : 64 * (h % 2) + 64,
                                                     512 * lg : 512 * (lg + 1)],
                                    ps[:], AF.Identity, bias=bias_sb[:, h : h + 1])
                    else:
                        wt = w_sb["v"]
                        for li in range(16):
                            ps2 = pvp.tile([128, 256], f32, name="ps2", tag="ps2")
                            for f in range(4):
                                nc.tensor.matmul(
                                    ps2[:],
                                    xT[f][:, 128 * li : 128 * (li + 1)],
                                    wt[:, 256 * f : 256 * (f + 1)],
                                    start=(f == 0), stop=(f == 3))
                            nc.vector.tensor_tensor(
                                out=vh_all[:, li, :], in0=ps2[:], in1=bv_sb[:],
                                op=Alu.add)

            # ---------- Phase B: attention per unit ----------
            outT = [projp.tile([DH, L], f32, name=f"outT{u}") for u in range(4)]
            with (
                tc.tile_pool(name="s_ps", bufs=2, space="PSUM") as sp,
                tc.tile_pool(name="at_ps", bufs=2, space="PSUM") as atp,
                tc.tile_pool(name="av_ps", bufs=2, space="PSUM") as avp,
                tc.tile_pool(name="e", bufs=6) as ep,
                tc.tile_pool(name="attT", bufs=3) as attTp,
                tc.tile_pool(name="sm", bufs=8) as smp,
            ):
                for u in range(4):
                    p, r = u // 2, u % 2
                    qpt, kpt = qp[p], kp[p]
                    for qg in range(4):
                        e_tiles = []
                        for qc in range(4):
                            qq = 4 * qg + qc
                            e_t = ep.tile([128, L], f32, name="e_t", tag="e")
                            den2 = smp.tile([128, 2], f32, name="den2", tag="sm")
                            for sh in range(2):
                                s_ps = sp.tile([128, 1024], f32, name="s_ps", tag="s")
                                for kb in range(2):
                                    nc.tensor.matmul(
                                        s_ps[:, 512 * kb : 512 * (kb + 1)],
                                        qpt[64 * r : 64 * r + 64, 128 * qq : 128 * (qq + 1)],
                                        kpt[64 * r : 64 * r + 64,
                                            1024 * sh + 512 * kb : 1024 * sh + 512 * (kb + 1)],
                                        start=True, stop=True)
                                nc.scalar.activation(
                                    e_t[:, 1024 * sh : 1024 * (sh + 1)], s_ps[:],
                                    AF.Exp, scale=0.125,
                                    accum_out=den2[:, sh : sh + 1])
                            dsum = smp.tile([128, 1], f32, name="dsum", tag="sm")
                            rec = smp.tile([128, 1], f32, name="rec", tag="sm")
                            nc.vector.tensor_tensor(
                                out=dsum[:], in0=den2[:, 0:1], in1=den2[:, 1:2],
                                op=Alu.add)
                            nc.vector.reciprocal(rec[:], dsum[:])
                            nc.vector.tensor_scalar_mul(e_t[:], e_t[:], rec[:, 0:1])
                            nc.sync.dma_start(
                                attp[u, 128 * qq : 128 * (qq + 1), :], e_t[:])
                            e_tiles.append(e_t)
                        av_ps = avp.tile([DH, 512], f32, name="av_ps", tag="av")
                        for kc in range(16):
                            tp2 = atp.tile([128, 512], f32, name="tp2", tag="at")
                            for qc in range(4):
                                nc.tensor.transpose(
                                    tp2[:, 128 * qc : 128 * (qc + 1)],
                                    e_tiles[qc][:, 128 * kc : 128 * (kc + 1)],
                                    ident[:])
                            attT = attTp.tile([128, 512], f32, name="attT", tag="attT")
                            if kc % 2 == 0:
                                nc.vector.tensor_copy(attT[:], tp2[:])
                            else:
                                nc.scalar.copy(attT[:], tp2[:])
                            nc.tensor.matmul(
                                av_ps[:], vh_all[:, kc, 64 * u : 64 * (u + 1)],
                                attT[:], start=(kc == 0), stop=(kc == 15))
                        nc.vector.tensor_copy(
                            outT[u][:, 512 * qg : 512 * (qg + 1)], av_ps[:])

            # ---------- Phase C: output projection partial + RS + LN ----------
            partial = dramp.tile([ROWS, D], f32)
            rsout = dramp.tile([RPC, D], f32)
            with (
                tc.tile_pool(name="pp_ps", bufs=2, space="PSUM") as ppp,
                tc.tile_pool(name="pp_sb", bufs=2) as ppsb,
                tc.tile_pool(name="ln", bufs=6) as lnp,
                tc.tile_pool(name="lns", bufs=12) as lnsp,
            ):
                for u in range(4):
                    for lg in range(4):
                        pp_sb = ppsb.tile([128, 2048], f32, name="pp_sb", tag="pp_sb")
                        for lc4 in range(4):
                            lc = 4 * lg + lc4
                            pp_ps = ppp.tile([128, 512], f32, name="pp_ps", tag="pp_ps")
                            nc.tensor.matmul(
                                pp_ps[:], outT[u][:, 128 * lc : 128 * (lc + 1)],
                                wo_sb[:], start=True, stop=True)
                            if lc4 % 2 == 0:
                                nc.vector.tensor_copy(
                                    pp_sb[:, 512 * lc4 : 512 * (lc4 + 1)], pp_ps[:])
                            else:
                                nc.scalar.copy(
                                    pp_sb[:, 512 * lc4 : 512 * (lc4 + 1)], pp_ps[:])
                        dst = partial[u * L + 512 * lg : u * L + 512 * (lg + 1), :]
                        nc.sync.dma_start(
                            dst.rearrange("(a p) o -> p a o", p=128),
                            pp_sb[:].rearrange("p (a o) -> p a o", a=4))
                nc.gpsimd.collective_compute(
                    "ReduceScatter", Alu.add,
                    replica_groups=[list(range(N_CORES))],
                    ins=[partial.opt()], outs=[rsout.opt()])
                for lt in range(8):
                    x_sb = lnp.tile([128, D], f32, name="x_sb", tag="ln")
                    rs_sb = lnp.tile([128, D], f32, name="rs_sb", tag="ln")
                    nc.sync.dma_start(x_sb[:], rsout[128 * lt : 128 * (lt + 1), :])
                    nc.sync.dma_start(rs_sb[:], resid[128 * lt : 128 * (lt + 1), :])
                    xr = lnp.tile([128, D], f32, name="xr", tag="ln")
                    nc.vector.tensor_tensor(out=xr[:], in0=x_sb[:], in1=rs_sb[:],
                                            op=Alu.add)
                    scr = lnp.tile([128, D], f32, name="scr", tag="ln")
                    ssum = lnsp.tile([128, 1], f32, name="ssum", tag="lns")
                    nc.scalar.activation(scr[:], xr[:], AF.Copy, accum_out=ssum[:])
                    nm = lnsp.tile([128, 1], f32, name="nm", tag="lns")
                    nc.vector.tensor_scalar(out=nm[:], in0=ssum[:],
                                            scalar1=-1.0 / D, scalar2=None,
                                            op0=Alu.mult)
                    xc = lnp.tile([128, D], f32, name="xc", tag="ln")
                    nc.vector.tensor_scalar_add(xc[:], xr[:], nm[:, 0:1])
                    scr2 = lnp.tile([128, D], f32, name="scr2", tag="ln")
                    ssq = lnsp.tile([128, 1], f32, name="ssq", tag="lns")
                    nc.scalar.activation(scr2[:], xc[:], AF.Square,
                                         accum_out=ssq[:])
                    veps = lnsp.tile([128, 1], f32, name="veps", tag="lns")
                    nc.vector.tensor_scalar(out=veps[:], in0=ssq[:],
                                            scalar1=1.0 / D, scalar2=LN_EPS,
                                            op0=Alu.mult, op1=Alu.add)
                    sqv = lnsp.tile([128, 1], f32, name="sqv", tag="lns")
                    nc.scalar.sqrt(sqv[:], veps[:])
                    r0 = lnsp.tile([128, 1], f32, name="r0", tag="lns")
                    nc.vector.reciprocal(r0[:], sqv[:])
                    # one Newton step: r = r0 * (1.5 - 0.5 * veps * r0^2)
                    t1 = lnsp.tile([128, 1], f32, name="t1", tag="lns")
                    nc.vector.tensor_tensor(out=t1[:], in0=r0[:], in1=r0[:],
                                            op=Alu.mult)
                    t2 = lnsp.tile([128, 1], f32, name="t2", tag="lns")
                    nc.vector.tensor_tensor(out=t2[:], in0=t1[:], in1=veps[:],
                                            op=Alu.mult)
                    t3 = lnsp.tile([128, 1], f32, name="t3", tag="lns")
                    nc.vector.tensor_scalar(out=t3[:], in0=t2[:], scalar1=-0.5,
                                            scalar2=1.5, op0=Alu.mult, op1=Alu.add)
                    rsq = lnsp.tile([128, 1], f32, name="rsq", tag="lns")
                    nc.vector.tensor_tensor(out=rsq[:], in0=r0[:], in1=t3[:],
                                            op=Alu.mult)
                    xn = lnp.tile([128, D], f32, name="xn", tag="ln")
                    nc.vector.tensor_scalar_mul(xn[:], xc[:], rsq[:, 0:1])
                    y1 = lnp.tile([128, D], f32, name="y1", tag="ln")
                    nc.vector.tensor_tensor(out=y1[:], in0=xn[:], in1=gam_sb[:],
                                            op=Alu.mult)
                    y = lnp.tile([128, D], f32, name="y", tag="ln")
                    nc.vector.tensor_tensor(out=y[:], in0=y1[:], in1=bet_sb[:],
                                            op=Alu.add)
                    nc.sync.dma_start(outp[128 * lt : 128 * (lt + 1), :], y[:])

    nc.compile()
    return nc


def build_in_maps(q, k, v, Wq, bq, Wk, bk, Wv, bv, Wo, bo, gamma, beta):
    q = np.asarray(q, np.float32)
    k = np.asarray(k, np.float32)
    v = np.asarray(v, np.float32)
    qf = np.ascontiguousarray(q.reshape(ROWS, D))
    gam_b = np.ascontiguousarray(np.broadcast_to(np.asarray(gamma, np.float32), (128, D)))
    bet_b = np.ascontiguousarray(np.broadcast_to(np.asarray(beta, np.float32), (128, D)))

    in_maps = []
    for c in range(N_CORES):
        b1 = c // 2
        h0 = 4 * (c % 2)
        hsl = slice(64 * h0, 64 * h0 + 256)
        m = {
            "xq": np.ascontiguousarray(q[b1]),
            "xk": np.ascontiguousarray(k[b1]),
            "xv": np.ascontiguousarray(v[b1]),
            "wqT": np.ascontiguousarray(np.asarray(Wq, np.float32)[hsl, :].T),
            "wkT": np.ascontiguousarray(np.asarray(Wk, np.float32)[hsl, :].T),
            "wvT": np.ascontiguousarray(np.asarray(Wv, np.float32)[hsl, :].T),
            "bq4": np.ascontiguousarray(np.asarray(bq, np.float32)[hsl].reshape(4, 64).T),
            "bk4": np.ascontiguousarray(np.asarray(bk, np.float32)[hsl].reshape(4, 64).T),
            "bv_b": np.ascontiguousarray(
                np.broadcast_to(np.asarray(bv, np.float32)[hsl], (128, 256))),
            "woT4": np.ascontiguousarray(np.asarray(Wo, np.float32)[:, 64 * c : 64 * (c + 1)].T),
            "resid": np.ascontiguousarray(
                qf[RPC * c : RPC * (c + 1)] + np.asarray(bo, np.float32)),
            "gam_b": gam_b,
            "bet_b": bet_b,
        }
        in_maps.append(m)
    return in_maps


def kernel(q, k, v, Wq, bq, Wk, bk, Wv, bv, Wo, bo, gamma, beta):
    from concourse.bass_utils import run_bass_kernel_spmd

    if "nc" not in _CACHE:
        _CACHE["nc"] = _build()
    nc = _CACHE["nc"]

    in_maps = build_in_maps(q, k, v, Wq, bq, Wk, bk, Wv, bv, Wo, bo, gamma, beta)
    res = run_bass_kernel_spmd(nc, in_maps, list(range(N_CORES)))
    raw_att = np.concatenate([res.results[c]["attp"] for c in range(N_CORES)], axis=0)
    out = np.concatenate([res.results[c]["outp"] for c in range(N_CORES)], axis=0)
    out = out.reshape(B, L, D)
    return (out, raw_att)
